# revision 40
# baseline (speedup 1.0000x reference)
"""Distributed causal self-attention kernel for 8 Trainium2 NeuronCores.

Problem: B=4, T=2048, C=1024, H=16 heads, D=64 head dim, fp32.
  qkv = x @ W_qkv.T + b_qkv; causal attention per head; out = attn @ W_proj.T + b_proj

Sharding (hybrid DP x TP, no on-device collectives):
  core c -> batch b = c//2 (data parallel), head group g = c%2 (8 heads each,
  tensor parallel). Each core computes a row-parallel *partial* projection
  output for its batch; the host sums the two partials per batch (the TP
  reduction) and adds b_proj.

Engine plan (per core), all matmul operands bf16:
  - Q^T/K^T produced in [j, T] bf16 (weight-stationary GEMM, 1/8 folded into
    Wq); scores_T = K^T.T @ Q^T with the two heads of a pair on disjoint
    64-row PE groups -> they execute CONCURRENTLY, so a pair's scores cost
    one head's cycles.
  - V in natural [T, j] bf16 in one flat region; a single shared ones-column
    block + per-k-tile V give a two-range stationary [ones|V_h] whose attn@V
    emits the softmax denominator and the unnormalized output in one pass.
  - exp on the Scalar engine; attention swept qc-outer/pair-inner with sweep
    order (1,2,3,0): the exp-heavy qc=3 sweep runs mid-kernel where qk/v
    fillers still exist, and the exp-light qc=0 sweep + its projection units
    form the tail, so the Scalar engine never gates the finish.
  - input DMA: 2D per-ct transfers priority-ordered across three issue
    queues (SP, Activation HWDGE, GpSimd SWDGE) so the first-pair deps
    (wqk jt0+jt4, x chunks 0,1) land by ~15us and the first qk matmuls
    start ~17us earlier than a single-queue load.
  - the proj fillers of the qc=1,3 sweeps are reserved for the final qc=0
    sweep, whose per-k-tile scores->exp->mask->attn@V chains are latency-
    bound.
  - diagonal-block causal masks on GpSimd (Pool), off the DVE.
  - output partials DMA out as bf16; the host sums partials in f32.
"""
import sys

if "/opt/trn_rl_repo" not in sys.path:
    sys.path.insert(0, "/opt/trn_rl_repo")

import ml_dtypes
import numpy as np

import concourse.bass as bass
import concourse.tile as tile
from concourse import bacc, mybir
from concourse.bass_utils import run_bass_kernel_spmd
from concourse.masks import make_upper_triangular

F32 = mybir.dt.float32
BF16 = mybir.dt.bfloat16

B, T, C = 4, 2048, 1024
H, D = 16, 64
HC = 8            # heads per core
P = 128
NCORES = 8
NT = T // P       # 16 k-tiles
NTC = T // 512    # 4 t-chunks / q-chunks

SW = (1, 2, 3, 0)  # sweep order: qc handled at each sweep index

_compiled = None


def build():
    nc = bacc.Bacc("TRN2", target_bir_lowering=False, debug=False,
                   num_devices=NCORES)
    xbf_ext = nc.declare_dram_parameter("xbf", [C, T], BF16, isOutput=False)
    wqk_ext = nc.declare_dram_parameter("wqk", [C, 1024], BF16, isOutput=False)
    bqk_ext = nc.declare_dram_parameter("bqk", [1024], F32, isOutput=False)
    wv_ext = nc.declare_dram_parameter("wv", [C, 512], BF16, isOutput=False)
    bv_ext = nc.declare_dram_parameter("bv", [512], F32, isOutput=False)
    wp_ext = nc.declare_dram_parameter("wp", [512, C], BF16, isOutput=False)
    bp_ext = nc.declare_dram_parameter("bp", [C], F32, isOutput=False)
    out_ext = nc.declare_dram_parameter("out", [C, T], BF16, isOutput=True)

    with tile.TileContext(nc, pool_alloc_mode="queue") as tc:
        _body(nc, tc, xbf_ext, wqk_ext, bqk_ext, wv_ext, bv_ext,
              wp_ext, bp_ext, out_ext)
    nc.compile()
    return nc


def _body(nc, tc, xbf_ext, wqk_ext, bqk_ext, wv_ext, bv_ext,
          wp_ext, bp_ext, out_ext):
    dma = nc.default_dma_engine      # SP HWDGE queue
    sdma = nc.scalar                 # Activation HWDGE queue (startup x loads)
    Exp = mybir.ActivationFunctionType.Exp

    from contextlib import ExitStack
    ctx = ExitStack()
    with ctx:
        singles = ctx.enter_context(tc.tile_pool(name="singles", bufs=1))
        qkt_pool = ctx.enter_context(tc.tile_pool(name="qkT", bufs=1))
        vpool = ctx.enter_context(tc.tile_pool(name="v", bufs=1))
        apool = ctx.enter_context(tc.tile_pool(name="attnT", bufs=1))
        ptpool = ctx.enter_context(tc.tile_pool(name="pt", bufs=4))
        rspool = ctx.enter_context(tc.tile_pool(name="rs", bufs=4))
        wp_pool = ctx.enter_context(tc.tile_pool(name="wp", bufs=1))
        opool = ctx.enter_context(tc.tile_pool(name="outs", bufs=2))
        psum = ctx.enter_context(tc.tile_pool(name="psum", bufs=1, space="PSUM"))
        xpool = ctx.enter_context(tc.tile_pool(name="x", bufs=1, side="right"))
        wqk_pool = ctx.enter_context(tc.tile_pool(name="wqk", bufs=1, side="right"))

        # ---- HAM warmup: dummy fp32 matmuls bridge the DMA wait so the PE
        # clock is at 8/8 when the first real matmuls issue.
        warm = rspool.tile([P, 512], F32, tag="rs", name="warm")
        nc.vector.memset(warm[:], 1.0)
        for i in range(8):
            wps = psum.tile([P, 512], F32, tag="mm", bufs=2, name=f"warm{i}")
            nc.tensor.matmul(wps[:], warm[:, 0:P], warm[:])

        # ---- flat SBUF input tiles ----
        wqk_all = wqk_pool.tile([P, 8 * 1024], BF16, tag="wqk", name="wqk")
        wv_all = wqk_pool.tile([P, 8 * 512], BF16, tag="wv", name="wv")
        xall = xpool.tile([P, 8 * T], BF16, tag="x", name="x")
        wp_all = wp_pool.tile([P, 4 * 1024], BF16, tag="wp", name="wp")

        # priority order (2D per-ct transfers, issued on three queues):
        #   SP:     wqk pair 0 (Q0|K0 cols, host-permuted adjacent) -> pairs
        #           1-3 -> wp
        #   ACT:    x chunks 0,1 -> biases -> x chunks 2,3
        #   GpSimd: wv
        # The first attention pair (qc=1) is gated only on pair-0 wqk + x01;
        # its 16 accumulation matmuls chase the per-ct x01 arrivals.
        for ct in range(8):
            sdma.dma_start(out=xall[:, ct * T:ct * T + 1024],
                           in_=xbf_ext[ct * P:(ct + 1) * P, 0:1024])
        for p_ in range(4):
            for ct in range(8):
                dma.dma_start(
                    out=wqk_all[:, ct * 1024 + 256 * p_:
                                ct * 1024 + 256 * p_ + 256],
                    in_=wqk_ext[ct * P:(ct + 1) * P, 256 * p_:256 * p_ + 256])

        # ---- constants / biases ----
        mask = singles.tile([P, P], BF16)       # m[tk,tq]=1 iff tq >= tk
        make_upper_triangular(nc, mask[:], val=1.0, diag=True)
        mask_b = bass.AP(tensor=mask[:].tensor, offset=mask[:].offset,
                         ap=[mask[:].ap[0], [0, 2], [1, P]])

        bqk_t = singles.tile([P, 8], F32)       # per-partition q/k biases
        sdma.dma_start(out=bqk_t[:], in_=bqk_ext[:].rearrange("(j p) -> p j", p=P))
        bv_b = singles.tile([P, 512], F32)      # v bias broadcast over partitions
        bv_src = bass.AP(tensor=bv_ext, offset=0, ap=[[0, P], [1, 512]])
        sdma.dma_start(out=bv_b[:], in_=bv_src)
        bv_b3 = bv_b[:].rearrange("p (h c) -> p h c", h=HC)
        bproj_t = singles.tile([P, 8], F32)
        sdma.dma_start(out=bproj_t[:], in_=bp_ext[:].rearrange("(m p) -> p m", p=P))

        # remaining loads, off the critical path
        for ct in range(8):
            nc.gpsimd.dma_start(out=wv_all[:, ct * 512:(ct + 1) * 512],
                                in_=wv_ext[ct * P:(ct + 1) * P, :])
        for ct in range(8):
            sdma.dma_start(out=xall[:, ct * T + 1024:ct * T + 2048],
                           in_=xbf_ext[ct * P:(ct + 1) * P, 1024:2048])
        for jt in range(4):
            dma.dma_start(out=wp_all[:, jt * 1024:(jt + 1) * 1024],
                          in_=wp_ext[jt * P:(jt + 1) * P, :])

        # ---- persistent SBUF state ----
        # qkT[jt]: j-tile jt of [Q^T | K^T] in [j, T] bf16; q jt 0..3 (pairs),
        # k jt 4..7. Within a j-tile: partitions 0-63 head 2p, 64-127 head 2p+1.
        qkT = [qkt_pool.tile([P, T], BF16, tag=f"qk{jt}", name=f"qkT{jt}")
               for jt in range(8)]
        # vall: per k-tile block of [8h x (ones(64) | V_h(64))]; the ones are
        # written once up front so v_tile only adds the bias
        vall = vpool.tile([P, NT * 1024], BF16, tag="vall", name="vall")
        vall4 = vall[:].rearrange("p (k h c) -> p k h c", k=NT, h=HC)
        nc.vector.memset(vall4[:, :, :, 0:64], 1.0)
        attnT = [apool.tile([P, T], BF16, tag=f"a{p_}", name=f"attnT{p_}")
                 for p_ in range(4)]

        def av_stat(kt, head):
            # stationary [ones(64) | V_head(64)] -> out partitions
            # 0:64 = denominator copies, 64:128 = outT rows
            return vall[:, kt * 1024 + head * 128:kt * 1024 + (head + 1) * 128]

        # ---- unit emitters ----
        def qk_unit(jt, tcn):
            # host-permuted wqk: pair p's Q tile at col 256p, K at 256p+128
            woff = 256 * (jt % 4) + 128 * (jt // 4)
            sl = slice(tcn * 512, (tcn + 1) * 512)
            ps = psum.tile([P, 512], F32, tag="mm", bufs=2,
                           name=f"psqk{jt}_{tcn}")
            for ct in range(8):
                nc.tensor.matmul(ps[:],
                                 wqk_all[:, ct * 1024 + woff:
                                         ct * 1024 + woff + P],
                                 xall[:, ct * T + tcn * 512:
                                      ct * T + (tcn + 1) * 512],
                                 start=(ct == 0), stop=(ct == 7))
            nc.vector.tensor_scalar_add(out=qkT[jt][:, sl], in0=ps[:],
                                        scalar1=bqk_t[:, jt:jt + 1])

        def v_tile(kt):
            psv = psum.tile([P, 512], F32, tag="mm", bufs=2, name=f"psv{kt}")
            for ct in range(8):
                nc.tensor.matmul(psv[:],
                                 xall[:, ct * T + kt * P:ct * T + (kt + 1) * P],
                                 wv_all[:, ct * 512:(ct + 1) * 512],
                                 start=(ct == 0), stop=(ct == 7))
            vt3 = vall[:, kt * 1024:(kt + 1) * 1024].rearrange(
                "p (h c) -> p h c", h=HC)
            nc.vector.tensor_add(vt3[:, :, 64:128],
                                 psv[:].rearrange("p (h c) -> p h c", h=HC),
                                 bv_b3)

        def proj_unit(tcn, mt, odma=None):
            sl = slice(tcn * 512, (tcn + 1) * 512)
            psp = psum.tile([P, 512], F32, tag="mm", bufs=2,
                            name=f"psp{mt}_{tcn}")
            for jt in range(4):
                nc.tensor.matmul(psp[:],
                                 wp_all[:, jt * 1024 + mt * P:
                                        jt * 1024 + (mt + 1) * P],
                                 attnT[jt][:, sl],
                                 start=(jt == 0), stop=(jt == 3))
            ot = opool.tile([P, 512], BF16, tag="ot", name=f"ot{mt}_{tcn}")
            nc.vector.tensor_scalar_add(out=ot[:], in0=psp[:],
                                        scalar1=bproj_t[:, mt:mt + 1])
            (odma or dma).dma_start(out=out_ext[mt * P:(mt + 1) * P, sl],
                                    in_=ot[:])

        # ---- filler stream: independent PE units woven into the attention
        # sweeps, deadline-scheduled so each unit lands just before its first
        # consumer instead of in a burst at sweep boundaries (which starves
        # the exp pipeline behind the in-order PE queue).
        TPT = 17          # time slots per (sweep, pair): kt 0..16
        END = 4 * 4 * TPT + 100   # strictly beyond any pull's now+lookahead

        def slot(i, p_, kt):
            return (i * 4 + p_) * TPT + kt

        # ready time for fillers touching x chunk c: chunks 0,1 land first;
        # 2,3 land mid sweep 0.
        def chunk_ready(c):
            return 0 if c < 2 else slot(0, 2, 0)

        fillers = []      # (deadline, ready, cost, fn)
        seen_k = set()
        for i, qc in enumerate(SW):
            # K units before Q at equal deadlines: the first pair's matmuls
            # then chase the per-ct x01 DMA arrivals in load order
            for c in range(qc + 1):
                if c in seen_k:
                    continue
                seen_k.add(c)
                for p_ in range(4):
                    fillers.append((slot(i, p_, max(0, 4 * c - 1)),
                                    chunk_ready(c), 2600,
                                    (lambda j, t: lambda: qk_unit(j, t))(4 + p_, c)))
            for p_ in range(4):
                fillers.append((slot(i, p_, 0), chunk_ready(qc), 2600,
                                (lambda j, t: lambda: qk_unit(j, t))(p_, qc)))
        for kt in range(NT):
            i_first = min(i for i, qc in enumerate(SW) if 4 * qc + 3 >= kt)
            fillers.append((slot(i_first, 0, kt), chunk_ready(kt // 4), 2600,
                            (lambda k: lambda: v_tile(k))(kt)))
        # proj hosting: proj(qc=2) weaves into sweep 2 (qc=3); proj(qc=1) and
        # proj(qc=3) are RESERVED as fillers for the latency-bound final qc=0
        # sweep (one forced per (pair, kt) pull there); proj(qc=0) drains at
        # the tail.
        for mt in range(8):
            fillers.append((slot(2, mt // 2, 16), slot(2, 0, 0), 1300,
                            (lambda m: lambda: proj_unit(2, m))(mt)))
        # kt offset +3 keeps the first deadline outside the pull lookahead of
        # the preceding sweep's final pulls (its attnT writer). proj(1) only
        # needs sweep 0's attnT, so its budget-ready opens at sweep 2 and can
        # fill the ACT-bound end of the qc=3 sweep.
        tail_units = [(qcq, mt) for mt in range(8) for qcq in (3, 1)]
        for u, (qcq, mt) in enumerate(tail_units):
            rd = slot(2, 0, 0) if qcq == 1 else slot(3, 0, 0)
            fillers.append((slot(3, u // 4, 3 + u % 4), rd, 1300,
                            (lambda t, m: lambda: proj_unit(t, m))(qcq, mt)))
        def proj_drain():
            # drain proj(0) 2-wide with jt ascending: each group's jt0-2
            # matmuls depend only on pairs 0-2 (normalized early), so they
            # execute during the final pair's exp/normalize chain and keep
            # the HAM clock warm; only jt3 waits on the last normalize.
            # Out-DMAs ride the then-idle ACT queue.
            for g in range(4):
                mts = (2 * g, 2 * g + 1)
                psps = [psum.tile([P, 512], F32, tag="mm", bufs=2,
                                  name=f"pspd{mt}") for mt in mts]
                for jt in range(4):
                    for i, mt in enumerate(mts):
                        nc.tensor.matmul(
                            psps[i][:],
                            wp_all[:, jt * 1024 + mt * P:
                                   jt * 1024 + (mt + 1) * P],
                            attnT[jt][:, 0:512],
                            start=(jt == 0), stop=(jt == 3))
                for i, mt in enumerate(mts):
                    ot = opool.tile([P, 512], BF16, tag="ot", name=f"otd{mt}")
                    nc.vector.tensor_scalar_add(
                        out=ot[:], in0=psps[i][:],
                        scalar1=bproj_t[:, mt:mt + 1])
                    sdma.dma_start(out=out_ext[mt * P:(mt + 1) * P, 0:512],
                                   in_=ot[:])

        fillers.append((END, END, 9000, proj_drain))
        fillers.sort(key=lambda u: (u[0], u[1]))
        state = {"budget": 0.0}

        def pull(now, add):
            state["budget"] = min(state["budget"] + add, 9000.0)
            i = 0
            while i < len(fillers):
                dl, ready, cost, fn = fillers[i]
                forced = dl <= now + 2
                if forced or (ready <= now and state["budget"] >= cost):
                    fn()
                    if not forced:
                        state["budget"] -= cost
                    fillers.pop(i)
                elif dl > now + 2 and ready > now:
                    i += 1
                else:
                    break

        def emit_av(pso, p_, pt, o, kt, nkt):
            for h in range(2):
                head = 2 * p_ + h
                nc.tensor.matmul(pso[h][:, o:],
                                 av_stat(kt, head),
                                 pt[:, 512 * h + o:512 * (h + 1)],
                                 start=(kt == 0), stop=(kt == nkt - 1))

        def attn(p_, qc, si):
            qTt = qkT[p_]
            kTt = qkT[4 + p_]
            nkt = 4 * (qc + 1)
            pso = [psum.tile([P, 512], F32, tag="o", bufs=2,
                             name=f"pso{p_}_{qc}_{h}") for h in range(2)]
            pend = []
            for kt in range(nkt):
                pull(slot(si, p_, kt), 1.1 * (512 - max(0, kt * P - qc * 512)))
                o = max(0, kt * P - qc * 512)
                ss = psum.tile([P, 1024], F32, tag="s", bufs=2,
                               name=f"pss{p_}_{qc}_{kt}")
                for h in range(2):
                    lo = 64 * h
                    # the two heads sit on disjoint 64-row PE groups and
                    # execute concurrently
                    nc.tensor.matmul(
                        ss[:, 512 * h + o:512 * (h + 1)],
                        kTt[lo:lo + 64, kt * P:(kt + 1) * P],
                        qTt[lo:lo + 64, qc * 512 + o:(qc + 1) * 512])
                pt = ptpool.tile([P, 1024], BF16, tag="pt",
                                 name=f"pt{p_}_{qc}_{kt}")
                ss3 = ss[:].rearrange("p (h w) -> p h w", h=2)
                pt3 = pt[:].rearrange("p (h w) -> p h w", h=2)
                nc.scalar.activation(pt3[:, :, o:], ss3[:, :, o:], Exp)
                if kt >= 4 * qc:
                    nc.gpsimd.tensor_mul(pt3[:, :, o:o + P],
                                         pt3[:, :, o:o + P], mask_b)
                # av for k-tile kt issues two iterations later, so the PE
                # never reaches it before exp(kt) has drained
                if len(pend) >= 2:
                    emit_av(pso, p_, *pend.pop(0), nkt)
                pend.append((pt, o, kt))
            for pd in pend:
                emit_av(pso, p_, *pd, nkt)
            # normalize: pso rows 0:64 = denominator copies, 64:128 = outT
            # (reciprocal output must stay on its input partitions -- a
            # partition-shifted write breaks on hardware; the DMA moves it)
            # the very last pair's broadcast rides GpSimd (its masks are all
            # done by then) so it is not queued behind SP out-DMA waits
            bdma = nc.gpsimd if (qc == 0 and p_ == 3) else dma
            for h in range(2):
                rsb = rspool.tile([P, 512], F32, tag="rs",
                                  name=f"rs{p_}_{qc}_{h}")
                nc.vector.reciprocal_approx_fast(rsb[0:64, :], pso[h][0:64, :])
                bdma.dma_start(out=rsb[64:128, :], in_=rsb[0:64, :])
                lo = 64 * h
                nc.vector.tensor_mul(
                    attnT[p_][lo:lo + 64, qc * 512:(qc + 1) * 512],
                    pso[h][64:128, :], rsb[64:128, :])

        # ---- schedule: sweeps in SW order; qk rounds, v-tiles and proj
        # units arrive just-in-time through the deadline-driven pulls ----
        for si, qc in enumerate(SW):
            for p_ in range(4):
                attn(p_, qc, si)
        while fillers:
            fillers.pop(0)[3]()


def shard_inputs(x, W_qkv, b_qkv, W_proj, b_proj):
    """Build the 8 per-core input maps (host-side sharding + layouts)."""
    x = np.asarray(x, np.float32)
    W_qkv = np.asarray(W_qkv, np.float32)
    b_qkv = np.asarray(b_qkv, np.float32)
    W_proj = np.asarray(W_proj, np.float32)
    b_proj = np.asarray(b_proj, np.float32)
    BF16NP = ml_dtypes.bfloat16

    in_maps = []
    for c in range(NCORES):
        b, g = c // 2, c % 2
        s = slice(512 * g, 512 * g + 512)
        Wq = W_qkv[0 * C:1 * C][s] * 0.125
        Wk = W_qkv[1 * C:2 * C][s]
        wqk = np.concatenate([Wq, Wk], 0).T          # [C, 1024]: q | k cols
        # permute j-columns so pair p's Q tile (128) and K tile (128) are
        # adjacent at cols 256p — one contiguous DMA covers a pair's weights
        perm = np.concatenate(
            [np.r_[128 * p:128 * p + 128, 512 + 128 * p:512 + 128 * p + 128]
             for p in range(4)])
        wqk = np.ascontiguousarray(wqk[:, perm])
        bq = b_qkv[0 * C:1 * C][s] * 0.125
        bk = b_qkv[1 * C:2 * C][s]
        xT = x[b].T
        in_maps.append({
            "xbf": np.ascontiguousarray(xT).astype(BF16NP),
            "wqk": wqk.astype(BF16NP),
            "bqk": np.ascontiguousarray(np.concatenate([bq, bk])),
            "wv": np.ascontiguousarray(W_qkv[2 * C:3 * C][s].T).astype(BF16NP),
            "bv": np.ascontiguousarray(b_qkv[2 * C:3 * C][s]),
            "wp": np.ascontiguousarray(W_proj[:, s].T).astype(BF16NP),
            "bp": b_proj if g == 0 else np.zeros_like(b_proj),
        })
    return in_maps


def run(in_maps, trace=False):
    global _compiled
    if _compiled is None:
        _compiled = build()
    return run_bass_kernel_spmd(
        _compiled, in_maps, core_ids=list(range(NCORES)), trace=trace)


def kernel(x, W_qkv, b_qkv, W_proj, b_proj):
    in_maps = shard_inputs(x, W_qkv, b_qkv, W_proj, b_proj)
    res = run(in_maps)
    out = np.empty((B, T, C), np.float32)
    for b in range(B):
        partial = (res.results[2 * b]["out"].astype(np.float32)
                   + res.results[2 * b + 1]["out"].astype(np.float32))
        out[b] = partial.T
    return out


if __name__ == "__main__":
    rng = np.random.default_rng(0)
    xs = {
        "x": rng.standard_normal((B, T, C)).astype(np.float32),
        "W_qkv": (rng.standard_normal((3 * C, C)) / 32).astype(np.float32),
        "b_qkv": (rng.standard_normal(3 * C) * 0.02).astype(np.float32),
        "W_proj": (rng.standard_normal((C, C)) / 32).astype(np.float32),
        "b_proj": (rng.standard_normal(C) * 0.02).astype(np.float32),
    }
    out = kernel(**xs)
    print("out", out.shape, out.dtype, np.abs(out).mean())
